# revision 25
# baseline (speedup 1.0000x reference)
"""Trainium2 kernel for nn_BatchedDTW.

The reference's banded-DTW recurrence is
    R[i, j] = D[i-1, j-1] + min(R[i-1, j-1], R[i-1, j])
whose predecessors both have row i-1, so i increments on every path step and
j never decreases. A path (0,0) -> (T,T) therefore takes exactly T steps and
must raise j by T, i.e. every step is diagonal, giving exactly
    R[T, T] = sum_t D[t, t] = sum_t ||x_t - y_t||_2
for any window >= 0 (the diagonal always satisfies |i-j| <= w).
So the whole problem collapses to
    out = mean_{b,n} sum_t ||X[b,t,n,:] - Y[b,t,n,:]||_2
       = (1/(B*N)) * sum over all (b,t,n) rows of sqrt(sum_c (X-Y)^2),
a pure streaming reduction over the flattened (B*T*N, C) rows, which we
shard contiguously across the 8 cores (row order is irrelevant to a sum).

Raw Bass (no Tile): this walrus build rejects instructions carrying more
than one sync wait, which Tile's tail drain needs, so semaphores are
explicit; every wait is its own instruction.

Engine layout (style="hyb", default):
  SP  : input chunk DMAs (HWDGE ring qSP); x|y packed per chunk so each
        chunk is ONE DMA -> one sem per consumer wait
  DVE : streaming chunks: sub (square runs on ACT, reduce back on DVE,
        keeping DVE at 2 passes, under the DMA stream); the LAST chunk
        runs sub/square/reduce entirely on DVE in program order, removing
        both cross-engine hops from the critical tail
  ACT : squares for streaming chunks; final sqrt (+free-dim accumulate in
        the same instruction); then issues the 512-B output DMA from its
        own HWDGE ring (qAct) and waits for its completion
Other styles kept for benching: "act" (square on ACT for all chunks),
"dve" (everything on DVE), "dual" (input DMAs split across both HWDGE
rings).
"""

from contextlib import ExitStack

import numpy as np

import concourse.bass as bass
import concourse.mybir as mybir
from concourse.bass_utils import run_bass_kernel_spmd

N_CORES = 8
P = 128                       # SBUF partitions
C = 32                        # channels per row (innermost axis of X/Y)
B, T, N = 4, 512, 64
ROWS_TOTAL = B * T * N        # 131072 rows of length C
ROWS_PER_CORE = ROWS_TOTAL // N_CORES     # 16384
F = ROWS_PER_CORE * C // P    # 4096 f32 per partition (16 KiB)
NCH = 8                       # input chunks -> 0.5 MiB DMAs
GT = F // C                   # 128 rows (length-C groups) per partition

_nc_cache = None
_last_results = None  # BassKernelResults from the most recent run (for benching)


def _build(nch=NCH, repeat=1, style="hyb"):
    """Build the per-core program.

    repeat > 1 re-runs the whole pipeline on the same input (double-buffered
    SBUF, exact semaphore bookkeeping) purely so on-device time per pipeline
    iteration can be measured as a slope between two repeat counts; the
    graded kernel uses repeat=1.
    """
    assert style in ("dve", "act", "dual", "hyb")
    fc = F // nch          # chunk elems per partition (per x or y half)
    gpc = fc // C          # rows per chunk per partition
    nbuf = 2 if repeat > 1 else 1
    nc = bass.Bass()
    f32 = mybir.dt.float32
    z_ext = nc.declare_dram_parameter("z", [P, 2 * F], f32, isOutput=False)
    out_ext = nc.declare_dram_parameter("out", [P, 1], f32, isOutput=True)

    with ExitStack() as ctx:
        zt = ctx.enter_context(nc.sbuf_tensor([P, nbuf * 2 * F], f32))
        df = ctx.enter_context(nc.sbuf_tensor([P, nbuf * F], f32))
        sq = ctx.enter_context(nc.sbuf_tensor([P, nbuf * F], f32))
        gs = ctx.enter_context(nc.sbuf_tensor([P, nbuf * GT], f32))
        dist = ctx.enter_context(nc.sbuf_tensor([P, nbuf * GT], f32))
        acc = ctx.enter_context(nc.sbuf_tensor([P, nbuf], f32))
        # One sem per in-flight input DMA: concurrent DMAs can complete out
        # of order, so cumulative waits on one shared sem would be racy.
        # Across repeats, sems are reused from a ring of KSETS sets with
        # cumulative (monotone) targets; the nbuf-deep buffer recycling
        # guarantees no two DMAs on the same sem are ever in flight at once.
        KSETS = min(repeat, 4)
        zsems = [ctx.enter_context(nc.semaphore(f"zsem{r}_{i}"))
                 for r in range(KSETS) for i in range(nch)]
        vsem = ctx.enter_context(nc.semaphore("vsem"))
        asem = ctx.enter_context(nc.semaphore("asem"))
        osem = ctx.enter_context(nc.semaphore("osem"))
        block = ctx.enter_context(nc.Block())

        def zs(r, ch):
            return zsems[(r % KSETS) * nch + ch]

        def z_done(r):  # zs(r, ch) value once rep r's chunk-ch DMA landed
            return 16 * (r // KSETS + 1)

        def zoff(r):  # free-dim base of rep r's zt buffer
            return (r % nbuf) * 2 * F

        def foff(r):
            return (r % nbuf) * F

        builder = {"dve": _build_dve, "act": _build_act,
                   "dual": _build_dual, "hyb": _build_hyb}[style]
        builder(nc, block, repeat, nch, fc, gpc, nbuf,
                zt, df, sq, gs, dist, acc,
                z_ext, out_ext, zs, z_done, zoff, foff,
                vsem, asem, osem)
    return nc


def _build_dual(nc, block, repeat, nch, fc, gpc, nbuf,
                zt, df, sq, gs, dist, acc,
                z_ext, out_ext, zs, z_done, zoff, foff,
                vsem, asem, osem):
    """Like _build_dve but input chunks alternate between the two HWDGE
    rings (even -> SP/qSP, odd -> ACT/qAct) to test whether one ring caps
    the streaming bandwidth."""
    def v_sub_done(r, ch):
        return 3 * nch * r + 3 * ch + 1

    def v_red_done(r, ch):
        return 3 * nch * r + 3 * ch + 3

    def issue_dma(eng, r, ch):
        if r >= nbuf:
            eng.wait_ge(vsem, v_sub_done(r - nbuf, ch))
        eng.dma_start(
            out=zt[:, zoff(r) + ch * 2 * fc:zoff(r) + (ch + 1) * 2 * fc],
            in_=z_ext[:, ch * 2 * fc:(ch + 1) * 2 * fc],
        ).then_inc(zs(r, ch), 16)

    @block.sync
    def _(sync):
        for r in range(repeat):
            for ch in range(0, nch, 2):
                issue_dma(sync, r, ch)

    @block.vector
    def _(vector):
        for r in range(repeat):
            if r >= nbuf:
                vector.wait_ge(asem, r - nbuf + 1)
            for ch in range(nch):
                vector.wait_ge(zs(r, ch), z_done(r))
                lo, hi = foff(r) + ch * fc, foff(r) + (ch + 1) * fc
                vector.tensor_sub(
                    df[:, lo:hi],
                    zt[:, zoff(r) + ch * 2 * fc:zoff(r) + ch * 2 * fc + fc],
                    zt[:, zoff(r) + ch * 2 * fc + fc:
                       zoff(r) + (ch + 1) * 2 * fc],
                ).then_inc(vsem, 1)
                vector.tensor_mul(
                    sq[:, lo:hi], df[:, lo:hi], df[:, lo:hi]
                ).then_inc(vsem, 1)
                vector.tensor_reduce(
                    out=gs[:, (r % nbuf) * GT + ch * gpc:
                           (r % nbuf) * GT + (ch + 1) * gpc],
                    in_=sq[:, lo:hi].rearrange("p (g c) -> p g c", c=C),
                    axis=mybir.AxisListType.X,
                    op=mybir.AluOpType.add,
                ).then_inc(vsem, 1)

    @block.scalar
    def _(scalar):
        def sqrt_rep(r):
            scalar.wait_ge(vsem, v_red_done(r, nch - 1))
            scalar.activation(
                out=dist[:, (r % nbuf) * GT:(r % nbuf + 1) * GT],
                in_=gs[:, (r % nbuf) * GT:(r % nbuf + 1) * GT],
                func=mybir.ActivationFunctionType.Sqrt,
                accum_out=acc[:, r % nbuf:r % nbuf + 1],
            ).then_inc(asem, 1)

        # software-pipelined by one rep so ACT's DMA issues for rep r are
        # enqueued before sqrt(r-1) blocks the sequencer (a deadlock-free
        # order w.r.t. DVE's gs-WAR wait on asem)
        for r in range(repeat):
            for ch in range(1, nch, 2):
                issue_dma(scalar, r, ch)
            if r >= 1:
                sqrt_rep(r - 1)
        sqrt_rep(repeat - 1)
        scalar.wait_ge(asem, repeat)
        scalar.dma_start(
            out=out_ext[:],
            in_=acc[:, (repeat - 1) % nbuf:(repeat - 1) % nbuf + 1],
        ).then_inc(osem, 16)
        scalar.wait_ge(osem, 16)


def _build_dve(nc, block, repeat, nch, fc, gpc, nbuf,
               zt, df, sq, gs, dist, acc,
               z_ext, out_ext, zs, z_done, zoff, foff,
               vsem, asem, osem):
    # vsem: 3 DVE ops per chunk (sub, square, reduce)
    def v_sub_done(r, ch):
        return 3 * nch * r + 3 * ch + 1

    def v_red_done(r, ch):
        return 3 * nch * r + 3 * ch + 3

    @block.sync
    def _(sync):
        for r in range(repeat):
            for ch in range(nch):
                if r >= nbuf:
                    # WAR: rep r reuses rep r-nbuf's zt chunk; its sub
                    # must have consumed it
                    sync.wait_ge(vsem, v_sub_done(r - nbuf, ch))
                sync.dma_start(
                    out=zt[:, zoff(r) + ch * 2 * fc:
                           zoff(r) + (ch + 1) * 2 * fc],
                    in_=z_ext[:, ch * 2 * fc:(ch + 1) * 2 * fc],
                ).then_inc(zs(r, ch), 16)

    @block.vector
    def _(vector):
        for r in range(repeat):
            if r >= nbuf:
                # WAR: gs slot r%nbuf was read by sqrt of rep r-nbuf
                vector.wait_ge(asem, r - nbuf + 1)
            for ch in range(nch):
                vector.wait_ge(zs(r, ch), z_done(r))
                lo, hi = foff(r) + ch * fc, foff(r) + (ch + 1) * fc
                vector.tensor_sub(
                    df[:, lo:hi],
                    zt[:, zoff(r) + ch * 2 * fc:zoff(r) + ch * 2 * fc + fc],
                    zt[:, zoff(r) + ch * 2 * fc + fc:
                       zoff(r) + (ch + 1) * 2 * fc],
                ).then_inc(vsem, 1)
                vector.tensor_mul(
                    sq[:, lo:hi], df[:, lo:hi], df[:, lo:hi]
                ).then_inc(vsem, 1)
                vector.tensor_reduce(
                    out=gs[:, (r % nbuf) * GT + ch * gpc:
                           (r % nbuf) * GT + (ch + 1) * gpc],
                    in_=sq[:, lo:hi].rearrange("p (g c) -> p g c", c=C),
                    axis=mybir.AxisListType.X,
                    op=mybir.AluOpType.add,
                ).then_inc(vsem, 1)

    @block.scalar
    def _(scalar):
        for r in range(repeat):
            scalar.wait_ge(vsem, v_red_done(r, nch - 1))
            scalar.activation(
                out=dist[:, (r % nbuf) * GT:(r % nbuf + 1) * GT],
                in_=gs[:, (r % nbuf) * GT:(r % nbuf + 1) * GT],
                func=mybir.ActivationFunctionType.Sqrt,
                accum_out=acc[:, r % nbuf:r % nbuf + 1],
            ).then_inc(asem, 1)
        # self-wait: the HWDGE dma_start below reads acc written by the
        # activation above; the sequencer runs ahead of the compute pipe,
        # so order must be enforced via the sem
        scalar.wait_ge(asem, repeat)
        scalar.dma_start(
            out=out_ext[:],
            in_=acc[:, (repeat - 1) % nbuf:(repeat - 1) % nbuf + 1],
        ).then_inc(osem, 16)
        scalar.wait_ge(osem, 16)


def _build_hyb(nc, block, repeat, nch, fc, gpc, nbuf,
               zt, df, sq, gs, dist, acc,
               z_ext, out_ext, zs, z_done, zoff, foff,
               vsem, asem, osem):
    """Streaming chunks 0..nch-2 use the act layout (DVE sub -> ACT square
    -> DVE reduce: DVE stays at 2 passes, under the DMA stream). The LAST
    chunk runs sub/square/reduce entirely on DVE in program order, removing
    both cross-engine hops from the critical tail. ACT then does sqrt(+acc)
    and issues the output DMA from its own ring."""
    VR = 2 * nch + 1   # vsem ticks per rep
    AR = nch           # asem ticks per rep: nch-1 squares + 1 sqrt

    def v_sub_done(r, ch):   # ch <= nch-2
        return VR * r + ch + 1

    def v_red_done(r, ch):   # ch <= nch-2
        return VR * r + (nch - 1) + ch + 1

    def v_sub_last_done(r):
        return VR * r + 2 * nch - 1

    def v_red_last_done(r):
        return VR * (r + 1)

    def a_sq_done(r, ch):    # ch <= nch-2
        return AR * r + ch + 1

    def a_sqrt_done(r):
        return AR * (r + 1)

    @block.sync
    def _(sync):
        for r in range(repeat):
            for ch in range(nch):
                if r >= nbuf:
                    # WAR: rep r reuses rep r-nbuf's zt chunk
                    sync.wait_ge(vsem,
                                 v_sub_last_done(r - nbuf) if ch == nch - 1
                                 else v_sub_done(r - nbuf, ch))
                sync.dma_start(
                    out=zt[:, zoff(r) + ch * 2 * fc:
                           zoff(r) + (ch + 1) * 2 * fc],
                    in_=z_ext[:, ch * 2 * fc:(ch + 1) * 2 * fc],
                ).then_inc(zs(r, ch), 16)

    @block.vector
    def _(vector):
        def sub(r, ch):
            lo, hi = foff(r) + ch * fc, foff(r) + (ch + 1) * fc
            return vector.tensor_sub(
                df[:, lo:hi],
                zt[:, zoff(r) + ch * 2 * fc:zoff(r) + ch * 2 * fc + fc],
                zt[:, zoff(r) + ch * 2 * fc + fc:zoff(r) + (ch + 1) * 2 * fc],
            ).then_inc(vsem, 1)

        def red(r, ch):
            lo, hi = foff(r) + ch * fc, foff(r) + (ch + 1) * fc
            return vector.tensor_reduce(
                out=gs[:, (r % nbuf) * GT + ch * gpc:
                       (r % nbuf) * GT + (ch + 1) * gpc],
                in_=sq[:, lo:hi].rearrange("p (g c) -> p g c", c=C),
                axis=mybir.AxisListType.X,
                op=mybir.AluOpType.add,
            ).then_inc(vsem, 1)

        for r in range(repeat):
            for ch in range(nch - 1):
                vector.wait_ge(zs(r, ch), z_done(r))
                if r >= nbuf:
                    # WAR: df slot last read by square(r-nbuf, ch)
                    vector.wait_ge(asem, a_sq_done(r - nbuf, ch))
                sub(r, ch)
            for ch in range(nch - 1):
                vector.wait_ge(asem, a_sq_done(r, ch))
                red(r, ch)
            ch = nch - 1
            vector.wait_ge(zs(r, ch), z_done(r))
            sub(r, ch)
            lo, hi = foff(r) + ch * fc, foff(r) + (ch + 1) * fc
            vector.tensor_mul(
                sq[:, lo:hi], df[:, lo:hi], df[:, lo:hi]
            ).then_inc(vsem, 1)
            red(r, ch)

    @block.scalar
    def _(scalar):
        for r in range(repeat):
            for ch in range(nch - 1):
                scalar.wait_ge(vsem, v_sub_done(r, ch))
                scalar.square(
                    out=sq[:, foff(r) + ch * fc:foff(r) + (ch + 1) * fc],
                    in_=df[:, foff(r) + ch * fc:foff(r) + (ch + 1) * fc],
                ).then_inc(asem, 1)
            scalar.wait_ge(vsem, v_red_last_done(r))
            scalar.activation(
                out=dist[:, (r % nbuf) * GT:(r % nbuf + 1) * GT],
                in_=gs[:, (r % nbuf) * GT:(r % nbuf + 1) * GT],
                func=mybir.ActivationFunctionType.Sqrt,
                accum_out=acc[:, r % nbuf:r % nbuf + 1],
            ).then_inc(asem, 1)
        # self-wait before reading our own activation's output via HWDGE
        scalar.wait_ge(asem, a_sqrt_done(repeat - 1))
        scalar.dma_start(
            out=out_ext[:],
            in_=acc[:, (repeat - 1) % nbuf:(repeat - 1) % nbuf + 1],
        ).then_inc(osem, 16)
        scalar.wait_ge(osem, 16)


def _build_act(nc, block, repeat, nch, fc, gpc, nbuf,
               zt, df, sq, gs, dist, acc,
               z_ext, out_ext, zs, z_done, zoff, foff,
               vsem, asem, osem):
    # vsem: per rep, nch subs then nch reduces
    def v_sub_done(r, ch):
        return 2 * nch * r + ch + 1

    def v_red_done(r, ch):
        return 2 * nch * r + nch + ch + 1

    # asem: per rep, nch squares then one sqrt
    def a_sq_done(r, ch):
        return (nch + 1) * r + ch + 1

    def a_sqrt_done(r):
        return (nch + 1) * (r + 1)

    @block.sync
    def _(sync):
        for r in range(repeat):
            for ch in range(nch):
                if r >= nbuf:
                    sync.wait_ge(vsem, v_sub_done(r - nbuf, ch))
                sync.dma_start(
                    out=zt[:, zoff(r) + ch * 2 * fc:
                           zoff(r) + (ch + 1) * 2 * fc],
                    in_=z_ext[:, ch * 2 * fc:(ch + 1) * 2 * fc],
                ).then_inc(zs(r, ch), 16)

    @block.vector
    def _(vector):
        for r in range(repeat):
            for ch in range(nch):
                vector.wait_ge(zs(r, ch), z_done(r))
                if r >= nbuf:
                    # WAR: df slot last read by square(r-nbuf, ch)
                    vector.wait_ge(asem, a_sq_done(r - nbuf, ch))
                vector.tensor_sub(
                    df[:, foff(r) + ch * fc:foff(r) + (ch + 1) * fc],
                    zt[:, zoff(r) + ch * 2 * fc:zoff(r) + ch * 2 * fc + fc],
                    zt[:, zoff(r) + ch * 2 * fc + fc:
                       zoff(r) + (ch + 1) * 2 * fc],
                ).then_inc(vsem, 1)
            for ch in range(nch):
                vector.wait_ge(asem, a_sq_done(r, ch))
                vector.tensor_reduce(
                    out=gs[:, (r % nbuf) * GT + ch * gpc:
                           (r % nbuf) * GT + (ch + 1) * gpc],
                    in_=sq[:, foff(r) + ch * fc:foff(r) + (ch + 1) * fc]
                    .rearrange("p (g c) -> p g c", c=C),
                    axis=mybir.AxisListType.X,
                    op=mybir.AluOpType.add,
                ).then_inc(vsem, 1)

    @block.scalar
    def _(scalar):
        for r in range(repeat):
            for ch in range(nch):
                # sub(r, ch) done also implies the rep r-nbuf reduce that
                # last read this sq slot finished (WAR safe)
                scalar.wait_ge(vsem, v_sub_done(r, ch))
                scalar.square(
                    out=sq[:, foff(r) + ch * fc:foff(r) + (ch + 1) * fc],
                    in_=df[:, foff(r) + ch * fc:foff(r) + (ch + 1) * fc],
                ).then_inc(asem, 1)
            scalar.wait_ge(vsem, v_red_done(r, nch - 1))
            scalar.activation(
                out=dist[:, (r % nbuf) * GT:(r % nbuf + 1) * GT],
                in_=gs[:, (r % nbuf) * GT:(r % nbuf + 1) * GT],
                func=mybir.ActivationFunctionType.Sqrt,
                accum_out=acc[:, r % nbuf:r % nbuf + 1],
            ).then_inc(asem, 1)
        # self-wait before reading our own activation's output via HWDGE
        scalar.wait_ge(asem, a_sqrt_done(repeat - 1))
        scalar.dma_start(
            out=out_ext[:],
            in_=acc[:, (repeat - 1) % nbuf:(repeat - 1) % nbuf + 1],
        ).then_inc(osem, 16)
        scalar.wait_ge(osem, 16)


def pack_inputs(X, Y, nch=NCH):
    """(B,T,N,C) x2 -> per-core packed z arrays, chunk-interleaved x|y."""
    fc = F // nch
    X = np.asarray(X, dtype=np.float32).reshape(N_CORES, P, nch, fc)
    Y = np.asarray(Y, dtype=np.float32).reshape(N_CORES, P, nch, fc)
    Z = np.empty((N_CORES, P, nch, 2, fc), dtype=np.float32)
    Z[:, :, :, 0, :] = X
    Z[:, :, :, 1, :] = Y
    return Z.reshape(N_CORES, P, 2 * F)


def kernel(X, Y, window=None, **_):
    global _nc_cache
    Z = pack_inputs(X, Y)
    if _nc_cache is None:
        _nc_cache = _build()
    in_maps = [{"z": Z[k]} for k in range(N_CORES)]
    res = run_bass_kernel_spmd(_nc_cache, in_maps, list(range(N_CORES)))
    global _last_results
    _last_results = res
    partials = np.stack([r["out"] for r in res.results])  # (8, 128, 1)
    total = partials.astype(np.float64).sum()
    return np.float32(total / (B * N))


# revision 26
# speedup vs baseline: 1.6970x; 1.6970x over previous
"""Trainium2 kernel for nn_BatchedDTW.

The reference's banded-DTW recurrence is
    R[i, j] = D[i-1, j-1] + min(R[i-1, j-1], R[i-1, j])
whose predecessors both have row i-1, so i increments on every path step and
j never decreases. A path (0,0) -> (T,T) therefore takes exactly T steps and
must raise j by T, i.e. every step is diagonal, giving exactly
    R[T, T] = sum_t D[t, t] = sum_t ||x_t - y_t||_2
for any window >= 0 (the diagonal always satisfies |i-j| <= w).
So the whole problem collapses to
    out = mean_{b,n} sum_t ||X[b,t,n,:] - Y[b,t,n,:]||_2
       = (1/(B*N)) * sum over all (b,t,n) rows of sqrt(sum_c (X-Y)^2),
a pure streaming reduction over the flattened (B*T*N, C) rows, which we
shard contiguously across the 8 cores (row order is irrelevant to a sum).

Raw Bass (no Tile): this walrus build rejects instructions carrying more
than one sync wait, which Tile's tail drain needs, so semaphores are
explicit; every wait is its own instruction.

Engine layout (style="hyb", default):
  SP  : input chunk DMAs (HWDGE ring qSP); x|y packed per chunk so each
        chunk is ONE DMA -> one sem per consumer wait
  DVE : streaming chunks: sub (square runs on ACT, reduce back on DVE,
        keeping DVE at 2 passes, under the DMA stream); the LAST chunk
        runs sub/square/reduce entirely on DVE in program order, removing
        both cross-engine hops from the critical tail
  ACT : squares for streaming chunks; final sqrt (+free-dim accumulate in
        the same instruction); then issues the 512-B output DMA from its
        own HWDGE ring (qAct) and waits for its completion
Other styles kept for benching: "act" (square on ACT for all chunks),
"dve" (everything on DVE), "dual" (input DMAs split across both HWDGE
rings).
"""

from contextlib import ExitStack

import numpy as np

import concourse.bass as bass
import concourse.mybir as mybir
from concourse.bass_utils import run_bass_kernel_spmd

N_CORES = 8
P = 128                       # SBUF partitions
C = 32                        # channels per row (innermost axis of X/Y)
B, T, N = 4, 512, 64
ROWS_TOTAL = B * T * N        # 131072 rows of length C
ROWS_PER_CORE = ROWS_TOTAL // N_CORES     # 16384
F = ROWS_PER_CORE * C // P    # 4096 f32 per partition (16 KiB)
NCH = 8                       # input chunks -> 0.5 MiB DMAs
GT = F // C                   # 128 rows (length-C groups) per partition

_nc_cache = None
_last_results = None  # BassKernelResults from the most recent run (for benching)


def _build(nch=NCH, repeat=1, style="hyb"):
    """Build the per-core program.

    repeat > 1 re-runs the whole pipeline on the same input (double-buffered
    SBUF, exact semaphore bookkeeping) purely so on-device time per pipeline
    iteration can be measured as a slope between two repeat counts; the
    graded kernel uses repeat=1.
    """
    assert style in ("dve", "act", "dual", "hyb")
    fc = F // nch          # chunk elems per partition (per x or y half)
    gpc = fc // C          # rows per chunk per partition
    nbuf = 2 if repeat > 1 else 1
    nc = bass.Bass()
    f32 = mybir.dt.float32
    z_ext = nc.declare_dram_parameter("z", [P, 2 * F], f32, isOutput=False)
    out_ext = nc.declare_dram_parameter("out", [P, 1], f32, isOutput=True)

    with ExitStack() as ctx:
        zt = ctx.enter_context(nc.sbuf_tensor([P, nbuf * 2 * F], f32))
        df = ctx.enter_context(nc.sbuf_tensor([P, nbuf * F], f32))
        sq = ctx.enter_context(nc.sbuf_tensor([P, nbuf * F], f32))
        gs = ctx.enter_context(nc.sbuf_tensor([P, nbuf * GT], f32))
        dist = ctx.enter_context(nc.sbuf_tensor([P, nbuf * GT], f32))
        acc = ctx.enter_context(nc.sbuf_tensor([P, nbuf], f32))
        # One sem per in-flight input DMA: concurrent DMAs can complete out
        # of order, so cumulative waits on one shared sem would be racy.
        # Across repeats, sems are reused from a ring of KSETS sets with
        # cumulative (monotone) targets; the nbuf-deep buffer recycling
        # guarantees no two DMAs on the same sem are ever in flight at once.
        KSETS = min(repeat, 4)
        zsems = [ctx.enter_context(nc.semaphore(f"zsem{r}_{i}"))
                 for r in range(KSETS) for i in range(nch)]
        vsem = ctx.enter_context(nc.semaphore("vsem"))
        asem = ctx.enter_context(nc.semaphore("asem"))
        osem = ctx.enter_context(nc.semaphore("osem"))
        block = ctx.enter_context(nc.Block())

        def zs(r, ch):
            return zsems[(r % KSETS) * nch + ch]

        def z_done(r):  # zs(r, ch) value once rep r's chunk-ch DMA landed
            return 16 * (r // KSETS + 1)

        def zoff(r):  # free-dim base of rep r's zt buffer
            return (r % nbuf) * 2 * F

        def foff(r):
            return (r % nbuf) * F

        builder = {"dve": _build_dve, "act": _build_act,
                   "dual": _build_dual, "hyb": _build_hyb}[style]
        builder(nc, block, repeat, nch, fc, gpc, nbuf,
                zt, df, sq, gs, dist, acc,
                z_ext, out_ext, zs, z_done, zoff, foff,
                vsem, asem, osem)
    return nc


def _build_dual(nc, block, repeat, nch, fc, gpc, nbuf,
                zt, df, sq, gs, dist, acc,
                z_ext, out_ext, zs, z_done, zoff, foff,
                vsem, asem, osem):
    """Like _build_dve but input chunks alternate between the two HWDGE
    rings (even -> SP/qSP, odd -> ACT/qAct) to test whether one ring caps
    the streaming bandwidth."""
    def v_sub_done(r, ch):
        return 3 * nch * r + 3 * ch + 1

    def v_red_done(r, ch):
        return 3 * nch * r + 3 * ch + 3

    def issue_dma(eng, r, ch):
        if r >= nbuf:
            eng.wait_ge(vsem, v_sub_done(r - nbuf, ch))
        eng.dma_start(
            out=zt[:, zoff(r) + ch * 2 * fc:zoff(r) + (ch + 1) * 2 * fc],
            in_=z_ext[:, ch * 2 * fc:(ch + 1) * 2 * fc],
        ).then_inc(zs(r, ch), 16)

    @block.sync
    def _(sync):
        for r in range(repeat):
            for ch in range(0, nch, 2):
                issue_dma(sync, r, ch)

    @block.vector
    def _(vector):
        for r in range(repeat):
            if r >= nbuf:
                vector.wait_ge(asem, r - nbuf + 1)
            for ch in range(nch):
                vector.wait_ge(zs(r, ch), z_done(r))
                lo, hi = foff(r) + ch * fc, foff(r) + (ch + 1) * fc
                vector.tensor_sub(
                    df[:, lo:hi],
                    zt[:, zoff(r) + ch * 2 * fc:zoff(r) + ch * 2 * fc + fc],
                    zt[:, zoff(r) + ch * 2 * fc + fc:
                       zoff(r) + (ch + 1) * 2 * fc],
                ).then_inc(vsem, 1)
                vector.tensor_mul(
                    sq[:, lo:hi], df[:, lo:hi], df[:, lo:hi]
                ).then_inc(vsem, 1)
                vector.tensor_reduce(
                    out=gs[:, (r % nbuf) * GT + ch * gpc:
                           (r % nbuf) * GT + (ch + 1) * gpc],
                    in_=sq[:, lo:hi].rearrange("p (g c) -> p g c", c=C),
                    axis=mybir.AxisListType.X,
                    op=mybir.AluOpType.add,
                ).then_inc(vsem, 1)

    @block.scalar
    def _(scalar):
        def sqrt_rep(r):
            scalar.wait_ge(vsem, v_red_done(r, nch - 1))
            scalar.activation(
                out=dist[:, (r % nbuf) * GT:(r % nbuf + 1) * GT],
                in_=gs[:, (r % nbuf) * GT:(r % nbuf + 1) * GT],
                func=mybir.ActivationFunctionType.Sqrt,
                accum_out=acc[:, r % nbuf:r % nbuf + 1],
            ).then_inc(asem, 1)

        # software-pipelined by one rep so ACT's DMA issues for rep r are
        # enqueued before sqrt(r-1) blocks the sequencer (a deadlock-free
        # order w.r.t. DVE's gs-WAR wait on asem)
        for r in range(repeat):
            for ch in range(1, nch, 2):
                issue_dma(scalar, r, ch)
            if r >= 1:
                sqrt_rep(r - 1)
        sqrt_rep(repeat - 1)
        scalar.wait_ge(asem, repeat)
        scalar.dma_start(
            out=out_ext[:],
            in_=acc[:, (repeat - 1) % nbuf:(repeat - 1) % nbuf + 1],
        ).then_inc(osem, 16)
        scalar.wait_ge(osem, 16)


def _build_dve(nc, block, repeat, nch, fc, gpc, nbuf,
               zt, df, sq, gs, dist, acc,
               z_ext, out_ext, zs, z_done, zoff, foff,
               vsem, asem, osem):
    # vsem: 3 DVE ops per chunk (sub, square, reduce)
    def v_sub_done(r, ch):
        return 3 * nch * r + 3 * ch + 1

    def v_red_done(r, ch):
        return 3 * nch * r + 3 * ch + 3

    @block.sync
    def _(sync):
        for r in range(repeat):
            for ch in range(nch):
                if r >= nbuf:
                    # WAR: rep r reuses rep r-nbuf's zt chunk; its sub
                    # must have consumed it
                    sync.wait_ge(vsem, v_sub_done(r - nbuf, ch))
                sync.dma_start(
                    out=zt[:, zoff(r) + ch * 2 * fc:
                           zoff(r) + (ch + 1) * 2 * fc],
                    in_=z_ext[:, ch * 2 * fc:(ch + 1) * 2 * fc],
                ).then_inc(zs(r, ch), 16)

    @block.vector
    def _(vector):
        for r in range(repeat):
            if r >= nbuf:
                # WAR: gs slot r%nbuf was read by sqrt of rep r-nbuf
                vector.wait_ge(asem, r - nbuf + 1)
            for ch in range(nch):
                vector.wait_ge(zs(r, ch), z_done(r))
                lo, hi = foff(r) + ch * fc, foff(r) + (ch + 1) * fc
                vector.tensor_sub(
                    df[:, lo:hi],
                    zt[:, zoff(r) + ch * 2 * fc:zoff(r) + ch * 2 * fc + fc],
                    zt[:, zoff(r) + ch * 2 * fc + fc:
                       zoff(r) + (ch + 1) * 2 * fc],
                ).then_inc(vsem, 1)
                vector.tensor_mul(
                    sq[:, lo:hi], df[:, lo:hi], df[:, lo:hi]
                ).then_inc(vsem, 1)
                vector.tensor_reduce(
                    out=gs[:, (r % nbuf) * GT + ch * gpc:
                           (r % nbuf) * GT + (ch + 1) * gpc],
                    in_=sq[:, lo:hi].rearrange("p (g c) -> p g c", c=C),
                    axis=mybir.AxisListType.X,
                    op=mybir.AluOpType.add,
                ).then_inc(vsem, 1)

    @block.scalar
    def _(scalar):
        for r in range(repeat):
            scalar.wait_ge(vsem, v_red_done(r, nch - 1))
            scalar.activation(
                out=dist[:, (r % nbuf) * GT:(r % nbuf + 1) * GT],
                in_=gs[:, (r % nbuf) * GT:(r % nbuf + 1) * GT],
                func=mybir.ActivationFunctionType.Sqrt,
                accum_out=acc[:, r % nbuf:r % nbuf + 1],
            ).then_inc(asem, 1)
        # self-wait: the HWDGE dma_start below reads acc written by the
        # activation above; the sequencer runs ahead of the compute pipe,
        # so order must be enforced via the sem
        scalar.wait_ge(asem, repeat)
        scalar.dma_start(
            out=out_ext[:],
            in_=acc[:, (repeat - 1) % nbuf:(repeat - 1) % nbuf + 1],
        ).then_inc(osem, 16)
        scalar.wait_ge(osem, 16)


def _build_hyb(nc, block, repeat, nch, fc, gpc, nbuf,
               zt, df, sq, gs, dist, acc,
               z_ext, out_ext, zs, z_done, zoff, foff,
               vsem, asem, osem):
    """Streaming chunks 0..nch-2 use the act layout (DVE sub -> ACT square
    -> DVE reduce: DVE stays at 2 passes, under the DMA stream). The LAST
    chunk runs sub/square/reduce entirely on DVE in program order, removing
    both cross-engine hops from the critical tail. ACT then does sqrt(+acc)
    and issues the output DMA from its own ring."""
    VR = 2 * nch + 1   # vsem ticks per rep
    AR = nch           # asem ticks per rep: nch-1 squares + 1 sqrt

    def v_sub_done(r, ch):   # all ch: subs come first in DVE program order
        return VR * r + ch + 1

    def v_red_done(r, ch):   # ch <= nch-2
        return VR * r + nch + ch + 1

    def v_red_last_done(r):
        return VR * (r + 1)

    def a_sq_done(r, ch):    # ch <= nch-2
        return AR * r + ch + 1

    def a_sqrt_done(r):
        return AR * (r + 1)

    @block.sync
    def _(sync):
        for r in range(repeat):
            for ch in range(nch):
                if r >= nbuf:
                    # WAR: rep r reuses rep r-nbuf's zt chunk
                    sync.wait_ge(vsem, v_sub_done(r - nbuf, ch))
                sync.dma_start(
                    out=zt[:, zoff(r) + ch * 2 * fc:
                           zoff(r) + (ch + 1) * 2 * fc],
                    in_=z_ext[:, ch * 2 * fc:(ch + 1) * 2 * fc],
                ).then_inc(zs(r, ch), 16)

    @block.vector
    def _(vector):
        def sub(r, ch):
            lo, hi = foff(r) + ch * fc, foff(r) + (ch + 1) * fc
            return vector.tensor_sub(
                df[:, lo:hi],
                zt[:, zoff(r) + ch * 2 * fc:zoff(r) + ch * 2 * fc + fc],
                zt[:, zoff(r) + ch * 2 * fc + fc:zoff(r) + (ch + 1) * 2 * fc],
            ).then_inc(vsem, 1)

        def red(r, ch):
            lo, hi = foff(r) + ch * fc, foff(r) + (ch + 1) * fc
            return vector.tensor_reduce(
                out=gs[:, (r % nbuf) * GT + ch * gpc:
                       (r % nbuf) * GT + (ch + 1) * gpc],
                in_=sq[:, lo:hi].rearrange("p (g c) -> p g c", c=C),
                axis=mybir.AxisListType.X,
                op=mybir.AluOpType.add,
            ).then_inc(vsem, 1)

        for r in range(repeat):
            # all subs first (incl. last chunk): each is ready as soon as its
            # DMA lands, and early subs release zt slots for recycling
            for ch in range(nch):
                vector.wait_ge(zs(r, ch), z_done(r))
                if r >= nbuf and ch < nch - 1:
                    # WAR: df slot last read by square(r-nbuf, ch); the last
                    # chunk's df is read by mul(r-nbuf) on this engine
                    vector.wait_ge(asem, a_sq_done(r - nbuf, ch))
                sub(r, ch)
            for ch in range(nch - 1):
                vector.wait_ge(asem, a_sq_done(r, ch))
                red(r, ch)
            ch = nch - 1
            lo, hi = foff(r) + ch * fc, foff(r) + (ch + 1) * fc
            vector.tensor_mul(
                sq[:, lo:hi], df[:, lo:hi], df[:, lo:hi]
            ).then_inc(vsem, 1)
            red(r, ch)

    @block.scalar
    def _(scalar):
        for r in range(repeat):
            for ch in range(nch - 1):
                scalar.wait_ge(vsem, v_sub_done(r, ch))
                scalar.square(
                    out=sq[:, foff(r) + ch * fc:foff(r) + (ch + 1) * fc],
                    in_=df[:, foff(r) + ch * fc:foff(r) + (ch + 1) * fc],
                ).then_inc(asem, 1)
            scalar.wait_ge(vsem, v_red_last_done(r))
            scalar.activation(
                out=dist[:, (r % nbuf) * GT:(r % nbuf + 1) * GT],
                in_=gs[:, (r % nbuf) * GT:(r % nbuf + 1) * GT],
                func=mybir.ActivationFunctionType.Sqrt,
                accum_out=acc[:, r % nbuf:r % nbuf + 1],
            ).then_inc(asem, 1)
        # self-wait before reading our own activation's output via HWDGE
        scalar.wait_ge(asem, a_sqrt_done(repeat - 1))
        scalar.dma_start(
            out=out_ext[:],
            in_=acc[:, (repeat - 1) % nbuf:(repeat - 1) % nbuf + 1],
        ).then_inc(osem, 16)
        scalar.wait_ge(osem, 16)


def _build_act(nc, block, repeat, nch, fc, gpc, nbuf,
               zt, df, sq, gs, dist, acc,
               z_ext, out_ext, zs, z_done, zoff, foff,
               vsem, asem, osem):
    # vsem: per rep, nch subs then nch reduces
    def v_sub_done(r, ch):
        return 2 * nch * r + ch + 1

    def v_red_done(r, ch):
        return 2 * nch * r + nch + ch + 1

    # asem: per rep, nch squares then one sqrt
    def a_sq_done(r, ch):
        return (nch + 1) * r + ch + 1

    def a_sqrt_done(r):
        return (nch + 1) * (r + 1)

    @block.sync
    def _(sync):
        for r in range(repeat):
            for ch in range(nch):
                if r >= nbuf:
                    sync.wait_ge(vsem, v_sub_done(r - nbuf, ch))
                sync.dma_start(
                    out=zt[:, zoff(r) + ch * 2 * fc:
                           zoff(r) + (ch + 1) * 2 * fc],
                    in_=z_ext[:, ch * 2 * fc:(ch + 1) * 2 * fc],
                ).then_inc(zs(r, ch), 16)

    @block.vector
    def _(vector):
        for r in range(repeat):
            for ch in range(nch):
                vector.wait_ge(zs(r, ch), z_done(r))
                if r >= nbuf:
                    # WAR: df slot last read by square(r-nbuf, ch)
                    vector.wait_ge(asem, a_sq_done(r - nbuf, ch))
                vector.tensor_sub(
                    df[:, foff(r) + ch * fc:foff(r) + (ch + 1) * fc],
                    zt[:, zoff(r) + ch * 2 * fc:zoff(r) + ch * 2 * fc + fc],
                    zt[:, zoff(r) + ch * 2 * fc + fc:
                       zoff(r) + (ch + 1) * 2 * fc],
                ).then_inc(vsem, 1)
            for ch in range(nch):
                vector.wait_ge(asem, a_sq_done(r, ch))
                vector.tensor_reduce(
                    out=gs[:, (r % nbuf) * GT + ch * gpc:
                           (r % nbuf) * GT + (ch + 1) * gpc],
                    in_=sq[:, foff(r) + ch * fc:foff(r) + (ch + 1) * fc]
                    .rearrange("p (g c) -> p g c", c=C),
                    axis=mybir.AxisListType.X,
                    op=mybir.AluOpType.add,
                ).then_inc(vsem, 1)

    @block.scalar
    def _(scalar):
        for r in range(repeat):
            for ch in range(nch):
                # sub(r, ch) done also implies the rep r-nbuf reduce that
                # last read this sq slot finished (WAR safe)
                scalar.wait_ge(vsem, v_sub_done(r, ch))
                scalar.square(
                    out=sq[:, foff(r) + ch * fc:foff(r) + (ch + 1) * fc],
                    in_=df[:, foff(r) + ch * fc:foff(r) + (ch + 1) * fc],
                ).then_inc(asem, 1)
            scalar.wait_ge(vsem, v_red_done(r, nch - 1))
            scalar.activation(
                out=dist[:, (r % nbuf) * GT:(r % nbuf + 1) * GT],
                in_=gs[:, (r % nbuf) * GT:(r % nbuf + 1) * GT],
                func=mybir.ActivationFunctionType.Sqrt,
                accum_out=acc[:, r % nbuf:r % nbuf + 1],
            ).then_inc(asem, 1)
        # self-wait before reading our own activation's output via HWDGE
        scalar.wait_ge(asem, a_sqrt_done(repeat - 1))
        scalar.dma_start(
            out=out_ext[:],
            in_=acc[:, (repeat - 1) % nbuf:(repeat - 1) % nbuf + 1],
        ).then_inc(osem, 16)
        scalar.wait_ge(osem, 16)


def pack_inputs(X, Y, nch=NCH):
    """(B,T,N,C) x2 -> per-core packed z arrays, chunk-interleaved x|y."""
    fc = F // nch
    X = np.asarray(X, dtype=np.float32).reshape(N_CORES, P, nch, fc)
    Y = np.asarray(Y, dtype=np.float32).reshape(N_CORES, P, nch, fc)
    Z = np.empty((N_CORES, P, nch, 2, fc), dtype=np.float32)
    Z[:, :, :, 0, :] = X
    Z[:, :, :, 1, :] = Y
    return Z.reshape(N_CORES, P, 2 * F)


def kernel(X, Y, window=None, **_):
    global _nc_cache
    Z = pack_inputs(X, Y)
    if _nc_cache is None:
        _nc_cache = _build()
    in_maps = [{"z": Z[k]} for k in range(N_CORES)]
    res = run_bass_kernel_spmd(_nc_cache, in_maps, list(range(N_CORES)))
    global _last_results
    _last_results = res
    partials = np.stack([r["out"] for r in res.results])  # (8, 128, 1)
    total = partials.astype(np.float64).sum()
    return np.float32(total / (B * N))
